# revision 12
# baseline (speedup 1.0000x reference)
"""Trainium2 8-core tensor-parallel Llama3-style GQA attention layer.

Problem: B=1, S=2048, D=4096, H=32 Q heads, KVH=8 KV heads, HD=128,
interleaved-pair RoPE (theta=5e5), causal softmax, output projection.

Sharding (Megatron TP-8):
  - core c owns Q heads [4c..4c+3] and KV head c (GQA groups align exactly),
  - x is replicated (passed pre-transposed as xT so the d-contraction sits on
    partitions with no on-device transposes),
  - wq/wk rows are permuted per head (even pair-indices first, then odd) so the
    interleaved RoPE becomes a "rotate-half" that is partition-aligned; the
    permutation cancels inside the q.k dot product,
  - attention runs in transposed layout (scoresT[s2,s1]) so the attention
    output lands as attnT[e, s] which is exactly the layout the output
    projection needs. The two heads of a GQA half-group share one 2-bank PSUM
    score tile so a single (wider) Exp serves both heads; softmax denominators
    come from a Pool-engine running sum of the exp'd tiles followed by one
    short ones-vector matmul per head (instead of a 512-row matmul per block),
  - attnT (bf16) is AllGathered across cores in 2 two-chunk pieces (overlapped
    with compute), and each core computes a 512-wide slice of the output dim
    of wo (column-parallel) => no reduction collective needed.

kernel(**inputs) takes the FULL fp32 inputs and returns the FULL fp32 output.
"""

import sys

sys.path.insert(0, "/opt/trn_rl_repo")

import math

import numpy as np
import ml_dtypes

import concourse.bass as bass  # noqa: F401
import concourse.mybir as mybir
import concourse.tile as tile
from concourse import bacc
from concourse.bass_utils import run_bass_kernel_spmd
from concourse.masks import make_identity

bf16 = ml_dtypes.bfloat16
F32 = mybir.dt.float32
BF16 = mybir.dt.bfloat16

# Problem shapes (hardcoded per spec)
B, S, D = 1, 2048, 4096
H, KVH, HD = 32, 8, 128
NCORES = 8
HLOC = H // NCORES            # 4 q heads per core
ELOC = HLOC * HD              # 512 attn-out dims per core
NKO = D // 128                # 32 k-tiles of the d contraction
CHUNK = 512                   # s-chunk (matmul free dim / psum bank)
NCHUNK = S // CHUNK           # 4
NB = S // 128                 # 16 s2 blocks
SCALE = 1.0 / math.sqrt(HD)

_NC_CACHE = None


def _build():
    nc = bacc.Bacc(
        "TRN2",
        target_bir_lowering=False,
        debug=False,
        enable_asserts=True,
        num_devices=NCORES,
    )
    xT_e = nc.dram_tensor("xT", [D, S], BF16, kind="ExternalInput")
    wq_e = nc.dram_tensor("wqT", [D, ELOC], BF16, kind="ExternalInput")
    wk_e = nc.dram_tensor("wkT", [D, HD], BF16, kind="ExternalInput")
    wv_e = nc.dram_tensor("wvT", [D, HD], BF16, kind="ExternalInput")
    wo_e = nc.dram_tensor("woT", [D, ELOC], BF16, kind="ExternalInput")
    cos_e = nc.dram_tensor("cosT", [HD, S], BF16, kind="ExternalInput")
    sin_e = nc.dram_tensor("sinT", [HD, S], BF16, kind="ExternalInput")
    out_e = nc.dram_tensor("out", [ELOC, S], F32, kind="ExternalOutput")

    xT = xT_e.ap().rearrange("(ko p) s -> p ko s", p=128)       # [128, 32, 2048]
    wqT = wq_e.ap().rearrange("(ko p) m -> p ko m", p=128)      # [128, 32, 512]
    wkT = wk_e.ap().rearrange("(ko p) m -> p ko m", p=128)      # [128, 32, 128]
    wvT = wv_e.ap().rearrange("(ko p) m -> p ko m", p=128)
    woT = wo_e.ap().rearrange("(ko p) m -> p ko m", p=128)      # [128, 32, 512]

    rg = [list(range(NCORES))]

    with tile.TileContext(nc) as tc:
        with (
            tc.tile_pool(name="dram", bufs=1, space="DRAM") as dram_pool,
            tc.tile_pool(name="persist", bufs=1) as pp,
        ):
            # AllGather buffers: one 2-chunk AG for s-chunks 0+1 (its
            # staging completes earliest under the A/B chunk interleave),
            # then one per chunk for 2 and 3 so each triggers as soon as its
            # staging lands.
            AG_WIDTHS = [2 * CHUNK, CHUNK, CHUNK]
            ag_in = [
                dram_pool.tile([ELOC, w], BF16, name=f"ag_in{k}")
                for k, w in enumerate(AG_WIDTHS)
            ]
            ag_out = [
                dram_pool.tile(
                    [NCORES * ELOC, w], BF16, name=f"ag_out{k}",
                    addr_space="Shared",
                )
                for k, w in enumerate(AG_WIDTHS)
            ]

            def ag_slot(j):
                """(ag index, column offset) for s-chunk j."""
                return (0, j * CHUNK) if j < 2 else (j - 1, 0)

            # ---- small constants ----
            # band[p, c, u] = 1 iff u >= p + 384, duplicated at c=0,1 so a
            # head-pair exp tile can be masked with one physical (non-
            # broadcast) multiply.
            band = pp.tile([128, 2, 896], BF16)
            nc.gpsimd.memset(band[:], 1.0)
            for c in range(2):
                nc.gpsimd.affine_select(
                    out=band[:, c, :], in_=band[:, c, :],
                    compare_op=mybir.AluOpType.is_ge, fill=0.0,
                    base=-384, channel_multiplier=-1, pattern=[[1, 896]],
                )
            ones_sb = pp.tile([128, 1], BF16)
            nc.gpsimd.memset(ones_sb[:], 1.0)
            ident = pp.tile([128, 128], BF16)
            make_identity(nc, ident[:])

            cos_sb = pp.tile([128, S], BF16)
            sin_sb = pp.tile([128, S], BF16)

            # ---- persistent activations ----
            qsb = pp.tile([128, HLOC, S], BF16)     # roped qT per head
            ksb = pp.tile([128, S], BF16)           # roped kT
            vsb = pp.tile([128, NB, HD], BF16)      # v[s2-tile, :, hd]

            with (
                tc.tile_pool(name="wq", bufs=1) as wqp,
                tc.tile_pool(name="wkv", bufs=1) as wkvp,
                tc.tile_pool(name="xch", bufs=1) as xp,
                tc.tile_pool(name="rope", bufs=2) as rp,
                tc.tile_pool(name="pt", bufs=5) as ptp,
                tc.tile_pool(name="acc", bufs=2) as accp,
                tc.tile_pool(name="misc", bufs=2) as mp,
                tc.tile_pool(name="stage", bufs=3) as stp,
            ):
                def load_xchunk(j):
                    js = slice(j * CHUNK, (j + 1) * CHUNK)
                    xc_g = []
                    for g in range(4):
                        t = xp.tile(
                            [128, 8, CHUNK], BF16, tag=f"xc{g}", bufs=2,
                            name=f"xc{j}_{g}",
                        )
                        eng = nc.gpsimd if g % 2 else nc.sync
                        for s0 in range(0, 8, 4):
                            eng.dma_start(
                                t[:, s0:s0 + 4, :],
                                xT[:, 8 * g + s0:8 * g + s0 + 4, js],
                            )
                        xc_g.append(t)
                    return xc_g

                def grp_load(pool, dram_t, m, name, eng, nsplit=2):
                    tiles = []
                    for g in range(4):
                        t = pool.tile([128, 8, m], BF16, name=f"{name}{g}")
                        step = 8 // nsplit
                        for s0 in range(0, 8, step):
                            eng.dma_start(
                                t[:, s0:s0 + step, :],
                                dram_t[:, 8 * g + s0:8 * g + s0 + step, :],
                            )
                        tiles.append(t)
                    return tiles

                # DMA queue plan at startup: wk FIRST on gpsimd (the first
                # matmuls need it), x split sync/gpsimd, the 4MB wq stream on
                # the otherwise-idle ACT queue so neither x nor wk sits
                # behind it, cos/sin on sync right after x chunk 0.
                wk_g = grp_load(wkvp, wkT, HD, "wk", nc.gpsimd, nsplit=2)
                xc0_g = load_xchunk(0)
                wq_g = grp_load(wqp, wqT, ELOC, "wq", nc.scalar, nsplit=2)
                for g in range(4):
                    sl = slice(g * 512, (g + 1) * 512)
                    nc.sync.dma_start(cos_sb[:, sl], cos_e.ap()[:, sl])
                    nc.sync.dma_start(sin_sb[:, sl], sin_e.ap()[:, sl])
                wv_g = grp_load(wkvp, wvT, HD, "wv", nc.gpsimd, nsplit=2)

                def rope(dst01, src_ps, js):
                    """dst01: (ap_lo, ap_hi) bf16 targets [64, 512] each.
                    src_ps: [128, 512] psum holding permuted projection."""
                    tc_t = rp.tile([128, CHUNK], F32, tag="ropec")
                    ts_t = rp.tile([128, CHUNK], F32, tag="ropes")
                    sw_t = rp.tile([128, CHUNK], F32, tag="ropew")
                    nc.vector.tensor_mul(tc_t[:], src_ps[:], cos_sb[:, js])
                    nc.vector.tensor_mul(ts_t[:], src_ps[:], sin_sb[:, js])
                    nc.sync.dma_start(sw_t[0:64, :], ts_t[64:128, :])
                    nc.sync.dma_start(sw_t[64:128, :], ts_t[0:64, :])
                    nc.vector.tensor_sub(dst01[0], tc_t[0:64, :], sw_t[0:64, :])
                    nc.vector.tensor_add(dst01[1], tc_t[64:128, :], sw_t[64:128, :])

                # ---- phases A+B interleaved per s-chunk: projections for
                # chunk j, then attention for chunk j (which only needs
                # k/v chunks <= j). Attention staging therefore completes
                # ~150us earlier, hiding the ~55-65us AllGather latencies
                # entirely behind remaining projection/attention work.
                # One shared 8-bank PSUM pool: kq(2) + sc(2x2) + o(2); the
                # denominator tiles borrow the kq slots (A and B never use
                # them at the same time).
                SKEW = 2
                staging_last = {}
                with tc.tile_pool(name="psAB", bufs=1, space="PSUM") as ps:
                    for j in range(NCHUNK):
                        js = slice(j * CHUNK, (j + 1) * CHUNK)
                        xc_g = xc0_g if j == 0 else load_xchunk(j)

                        # -- A(j): k, v, then q projections (v first so its
                        # DMA transposes land before B(j)'s first PV) --
                        k_ps = ps.tile([128, CHUNK], F32, tag="kq", bufs=2,
                                       name=f"kps_{j}")
                        for ko in range(NKO):
                            nc.tensor.matmul(
                                k_ps[:],
                                wk_g[ko // 8][:, ko % 8, :],
                                xc_g[ko // 8][:, ko % 8, :],
                                start=(ko == 0), stop=(ko == NKO - 1),
                            )
                        rope((ksb[0:64, js], ksb[64:128, js]), k_ps, js)

                        v_ps = ps.tile([128, CHUNK], F32, tag="kq", bufs=2,
                                       name=f"vps_{j}")
                        for ko in range(NKO):
                            nc.tensor.matmul(
                                v_ps[:],
                                wv_g[ko // 8][:, ko % 8, :],
                                xc_g[ko // 8][:, ko % 8, :],
                                start=(ko == 0), stop=(ko == NKO - 1),
                            )
                        vT_sb = mp.tile([128, CHUNK], BF16, tag="vtsb",
                                        name=f"vt_{j}")
                        nc.scalar.activation(
                            vT_sb[:], v_ps[:], mybir.ActivationFunctionType.Copy
                        )
                        for t in range(4):
                            nc.sync.dma_start_transpose(
                                vsb[:, 4 * j + t, :],
                                vT_sb[:, t * 128:(t + 1) * 128],
                            )

                        for h in range(HLOC):
                            q_ps = ps.tile([128, CHUNK], F32, tag="kq", bufs=2,
                                           name=f"qps_{j}_{h}")
                            for ko in range(NKO):
                                nc.tensor.matmul(
                                    q_ps[:],
                                    wq_g[ko // 8][:, ko % 8, h * 128:(h + 1) * 128],
                                    xc_g[ko // 8][:, ko % 8, :],
                                    start=(ko == 0), stop=(ko == NKO - 1),
                                )
                            rope((qsb[0:64, h, js], qsb[64:128, h, js]), q_ps, js)

                        # -- B(j): attention for chunk j over s2-blocks
                        # 0..4(j+1)-1, heads in GQA half-group pairs --
                        nblk = 4 * (j + 1)
                        for hp in range(2):
                            ngrp = j + 1     # groups of 4 s2-blocks
                            o = {}
                            dn = {}
                            for hh in range(2):
                                o[hh] = ps.tile(
                                    [128, CHUNK], F32, tag="o", bufs=2,
                                    name=f"o_{j}_{hp}_{hh}",
                                )
                                dn[hh] = ps.tile(
                                    [128, CHUNK], F32, tag="kq", bufs=2,
                                    name=f"dn_{j}_{hp}_{hh}",
                                )
                            pts = {}
                            # per-4-block bf16 exp sums (short independent
                            # DVE chains; the denominator then needs only
                            # ngrp short accumulating matmuls per head)
                            grp = {}

                            def issue_sc(i, j=j, hp=hp, pts=pts, grp=grp):
                                # Diagonal blocks at offset t>=1 have columns
                                # < 128*t fully masked: trim the matmul/exp
                                # free dim to the valid range.
                                t = i - 4 * j
                                lo = 128 * t if t >= 1 else 0
                                sc = ps.tile(
                                    [128, 2, CHUNK], F32, tag="sc", bufs=2,
                                    name=f"sc_{j}_{hp}_{i}",
                                )
                                for hh in range(2):
                                    nc.tensor.matmul(
                                        sc[:, hh, lo:CHUNK],
                                        ksb[:, i * 128:(i + 1) * 128],
                                        qsb[:, 2 * hp + hh,
                                            j * CHUNK + lo:(j + 1) * CHUNK],
                                        start=True, stop=True,
                                    )
                                pt = ptp.tile(
                                    [128, 2, CHUNK], BF16, tag="pt",
                                    name=f"pt_{j}_{hp}_{i}",
                                )
                                nc.scalar.activation(
                                    pt[:, :, lo:CHUNK], sc[:, :, lo:CHUNK],
                                    mybir.ActivationFunctionType.Exp,
                                    scale=SCALE,
                                )
                                if t >= 0:  # diagonal block: zero s1 < s2
                                    nc.vector.tensor_mul(
                                        pt[:, :, lo:CHUNK], pt[:, :, lo:CHUNK],
                                        band[:, :, 384:896 - lo],
                                    )
                                g = i // 4
                                if i % 4 == 0:
                                    grp[g] = (pt, lo, None)
                                elif i % 4 == 1:
                                    # first add CREATES the group sum in a
                                    # fresh tile (pt tiles stay pristine for
                                    # their PV matmuls)
                                    pt0, lo0, _ = grp[g]
                                    acc = ptp.tile(
                                        [128, 2, CHUNK], BF16, tag="gacc",
                                        bufs=2, name=f"gacc_{j}_{hp}_{g}",
                                    )
                                    nc.vector.tensor_add(
                                        acc[:, :, lo:CHUNK],
                                        pt0[:, :, lo:CHUNK],
                                        pt[:, :, lo:CHUNK],
                                    )
                                    if lo > lo0:
                                        nc.vector.tensor_copy(
                                            acc[:, :, lo0:lo],
                                            pt0[:, :, lo0:lo],
                                        )
                                    grp[g] = (pt0, lo0, acc)
                                else:
                                    _, _, acc = grp[g]
                                    nc.vector.tensor_add(
                                        acc[:, :, lo:CHUNK],
                                        acc[:, :, lo:CHUNK],
                                        pt[:, :, lo:CHUNK],
                                    )
                                pts[i] = (pt, lo)

                            for i in range(min(SKEW, nblk)):
                                issue_sc(i)
                            for i in range(nblk):
                                if i + SKEW < nblk:
                                    issue_sc(i + SKEW)
                                pt, lo = pts.pop(i)
                                # both heads' PV share lhsT=vsb[:,i,:]: back
                                # to back with one weight tile
                                for hh in range(2):
                                    nc.tensor.matmul(
                                        o[hh][:, lo:CHUNK], vsb[:, i, :],
                                        pt[:, hh, lo:CHUNK],
                                        start=(i == 0), stop=(i == nblk - 1),
                                    )
                                if i % 4 == 3:
                                    g = i // 4
                                    _, _, acc = grp.pop(g)
                                    for hh in range(2):
                                        nc.tensor.matmul(
                                            dn[hh][0:1, :], ones_sb[:],
                                            acc[:, hh, :],
                                            start=(g == 0),
                                            stop=(g == ngrp - 1),
                                        )
                            # evacuate psum fast, then normalize + stage
                            # (staging on the gpsimd DMA queue: the sync
                            # queue carries x/agsb/wo bulk loads)
                            for hh in range(2):
                                h = 2 * hp + hh
                                oun = mp.tile([128, CHUNK], F32, tag="oun",
                                              name=f"oun_{j}_{h}")
                                nc.vector.tensor_copy(oun[:], o[hh][:])
                                recip = mp.tile([1, CHUNK], F32, tag="recip",
                                                name=f"rc_{j}_{h}")
                                nc.vector.reciprocal_approx_fast(
                                    recip[:], dn[hh][0:1, :]
                                )
                                rb = mp.tile([128, CHUNK], F32, tag="rb",
                                             name=f"rb_{j}_{h}")
                                nc.gpsimd.partition_broadcast(rb[:], recip[:])
                                att = stp.tile([128, CHUNK], BF16, tag="att",
                                               name=f"att_{j}_{h}")
                                nc.vector.tensor_mul(att[:], oun[:], rb[:])
                                k, co = ag_slot(j)
                                last_attn_inst = nc.gpsimd.dma_start(
                                    ag_in[k][h * 128:(h + 1) * 128,
                                             co:co + CHUNK],
                                    att[:],
                                )
                                staging_last[j] = last_attn_inst
                        if j >= 1:
                            k = ag_slot(j)[0]
                            nc.gpsimd.collective_compute(
                                "AllGather",
                                mybir.AluOpType.bypass,
                                replica_groups=rg,
                                ins=[ag_in[k][:].opt()],
                                outs=[ag_out[k][:].opt()],
                            )

            # ---- phase C: output projection (column-parallel) ----
            with (
                tc.tile_pool(name="wo", bufs=1) as wop,
                tc.tile_pool(name="ag", bufs=2) as agp,
                tc.tile_pool(name="ost", bufs=3) as ostp,
                tc.tile_pool(name="psC", bufs=2, space="PSUM") as psc,
            ):
                wo_sb = wop.tile([128, NKO, ELOC], BF16)
                for g in range(8):
                    ko = slice(4 * g, 4 * g + 4)
                    nc.sync.dma_start(wo_sb[:, ko, :], woT[:, ko, :])
                first_wo = True
                for j in range(NCHUNK):
                    js = slice(j * CHUNK, (j + 1) * CHUNK)
                    k, co = ag_slot(j)
                    agt = ag_out[k][:].rearrange("(ko p) s -> p ko s", p=128)
                    # split the gathered-attn load into 4 e-groups so the first
                    # wo matmuls start after ~1MB instead of 4MB of DMA
                    aggrp = []
                    for g in range(4):
                        agsb = agp.tile(
                            [128, 8, CHUNK], BF16, tag=f"agsb{g}", bufs=2
                        )
                        # These wait on their AllGather's semaphore inside
                        # the sync FIFO; nothing latency-critical sits behind
                        # them there (staging runs on gpsimd), and chunks 0/1
                        # prefetch during B(2)/B(3) once AG0 lands.
                        nc.sync.dma_start(
                            agsb[:, 0:4, :],
                            agt[:, 8 * g:8 * g + 4, co:co + CHUNK],
                        )
                        nc.sync.dma_start(
                            agsb[:, 4:8, :],
                            agt[:, 8 * g + 4:8 * g + 8, co:co + CHUNK],
                        )
                        aggrp.append(agsb)
                    for t in range(4):
                        wo_ps = psc.tile([128, CHUNK], F32, tag="wo")
                        for ko in range(NKO):
                            mm = nc.tensor.matmul(
                                wo_ps[:],
                                wo_sb[:, ko, t * 128:(t + 1) * 128],
                                aggrp[ko // 8][:, ko % 8, :],
                                start=(ko == 0), stop=(ko == NKO - 1),
                            )
                            if first_wo:
                                # keep the PE stream ordered: all attention
                                # before any wo
                                tile.add_dep_helper(
                                    mm.ins, last_attn_inst.ins, sync=False,
                                    reason="attention before wo on PE",
                                )
                                first_wo = False
                        osb = ostp.tile([128, CHUNK], F32, tag="osb")
                        if j == 3 and t == 3:
                            # last tile: evacuate on the idle DVE (skips the
                            # ACT queue) and fan the write across both DMA
                            # pools to shorten the kernel tail
                            nc.vector.tensor_copy(osb[:], wo_ps[:])
                            engs = [nc.sync, nc.gpsimd, nc.sync, nc.gpsimd]
                            for q in range(4):
                                c0 = 128 * q
                                engs[q].dma_start(
                                    out_e.ap()[t * 128:(t + 1) * 128,
                                               js.start + c0:js.start + c0 + 128],
                                    osb[:, c0:c0 + 128],
                                )
                        else:
                            nc.scalar.activation(
                                osb[:], wo_ps[:],
                                mybir.ActivationFunctionType.Copy
                            )
                            nc.gpsimd.dma_start(
                                out_e.ap()[t * 128:(t + 1) * 128,
                                           js.start:js.start + 256],
                                osb[:, 0:256],
                            )
                            nc.gpsimd.dma_start(
                                out_e.ap()[t * 128:(t + 1) * 128,
                                           js.start + 256:js.stop],
                                osb[:, 256:CHUNK],
                            )

    nc.compile()
    return nc


def _get_nc():
    global _NC_CACHE
    if _NC_CACHE is None:
        _NC_CACHE = _build()
    return _NC_CACHE


_PERM = np.concatenate([np.arange(0, HD, 2), np.arange(1, HD, 2)])


def _prep_inputs(x, freqs_cos, freqs_sin, wq, wk, wv, wo):
    xT = np.ascontiguousarray(x.reshape(S, D).T.astype(bf16))
    cosT = np.ascontiguousarray(
        np.concatenate([freqs_cos.T, freqs_cos.T], axis=0).astype(bf16)
    )
    sinT = np.ascontiguousarray(
        np.concatenate([freqs_sin.T, freqs_sin.T], axis=0).astype(bf16)
    )
    in_maps = []
    for c in range(NCORES):
        heads = range(HLOC * c, HLOC * (c + 1))
        wq_c = np.concatenate(
            [wq[h * HD:(h + 1) * HD][_PERM] for h in heads], axis=0
        )  # [512, D] permuted
        wqT_c = np.ascontiguousarray(wq_c.T.astype(bf16))
        wk_c = wk[c * HD:(c + 1) * HD][_PERM]
        wkT_c = np.ascontiguousarray(wk_c.T.astype(bf16))
        wv_c = wv[c * HD:(c + 1) * HD]
        wvT_c = np.ascontiguousarray(wv_c.T.astype(bf16))
        woT_c = np.ascontiguousarray(wo[c * ELOC:(c + 1) * ELOC, :].T.astype(bf16))
        in_maps.append(
            {
                "xT": xT,
                "wqT": wqT_c,
                "wkT": wkT_c,
                "wvT": wvT_c,
                "woT": woT_c,
                "cosT": cosT,
                "sinT": sinT,
            }
        )
    return in_maps


def _run(in_maps, trace=False, trace_cores=None):
    nc = _get_nc()
    return run_bass_kernel_spmd(
        nc,
        in_maps,
        list(range(NCORES)),
        trace=trace,
        trace_cores=trace_cores,
    )


def kernel(x, freqs_cos, freqs_sin, wq, wk, wv, wo):
    x = np.asarray(x, dtype=np.float32)
    in_maps = _prep_inputs(
        x,
        np.asarray(freqs_cos, np.float32),
        np.asarray(freqs_sin, np.float32),
        np.asarray(wq, np.float32),
        np.asarray(wk, np.float32),
        np.asarray(wv, np.float32),
        np.asarray(wo, np.float32),
    )
    res = _run(in_maps)
    out = np.empty((S, D), dtype=np.float32)
    for c in range(NCORES):
        out[:, c * ELOC:(c + 1) * ELOC] = np.asarray(
            res.results[c]["out"], dtype=np.float32
        ).T
    return out.reshape(B, S, D)


# revision 14
# speedup vs baseline: 1.0490x; 1.0490x over previous
"""Trainium2 8-core tensor-parallel Llama3-style GQA attention layer.

Problem: B=1, S=2048, D=4096, H=32 Q heads, KVH=8 KV heads, HD=128,
interleaved-pair RoPE (theta=5e5), causal softmax, output projection.

Sharding (Megatron TP-8):
  - core c owns Q heads [4c..4c+3] and KV head c (GQA groups align exactly),
  - x is replicated (passed pre-transposed as xT so the d-contraction sits on
    partitions with no on-device transposes),
  - wq/wk rows are permuted per head (even pair-indices first, then odd) so the
    interleaved RoPE becomes a "rotate-half" that is partition-aligned; the
    permutation cancels inside the q.k dot product,
  - attention runs in transposed layout (scoresT[s2,s1]) so the attention
    output lands as attnT[e, s] which is exactly the layout the output
    projection needs. The two heads of a GQA half-group share one 2-bank PSUM
    score tile so a single (wider) Exp serves both heads; softmax denominators
    come from a Pool-engine running sum of the exp'd tiles followed by one
    short ones-vector matmul per head (instead of a 512-row matmul per block),
  - attnT (bf16) is AllGathered across cores in 2 two-chunk pieces (overlapped
    with compute), and each core computes a 512-wide slice of the output dim
    of wo (column-parallel) => no reduction collective needed.

kernel(**inputs) takes the FULL fp32 inputs and returns the FULL fp32 output.
"""

import sys

sys.path.insert(0, "/opt/trn_rl_repo")

import math

import numpy as np
import ml_dtypes

import concourse.bass as bass  # noqa: F401
import concourse.mybir as mybir
import concourse.tile as tile
from concourse import bacc
from concourse.bass_utils import run_bass_kernel_spmd
from concourse.masks import make_identity

bf16 = ml_dtypes.bfloat16
F32 = mybir.dt.float32
BF16 = mybir.dt.bfloat16

# Problem shapes (hardcoded per spec)
B, S, D = 1, 2048, 4096
H, KVH, HD = 32, 8, 128
NCORES = 8
HLOC = H // NCORES            # 4 q heads per core
ELOC = HLOC * HD              # 512 attn-out dims per core
NKO = D // 128                # 32 k-tiles of the d contraction
CHUNK = 512                   # s-chunk (matmul free dim / psum bank)
NCHUNK = S // CHUNK           # 4
NB = S // 128                 # 16 s2 blocks
SCALE = 1.0 / math.sqrt(HD)

_NC_CACHE = None


def _build():
    nc = bacc.Bacc(
        "TRN2",
        target_bir_lowering=False,
        debug=False,
        enable_asserts=True,
        num_devices=NCORES,
    )
    xT_e = nc.dram_tensor("xT", [D, S], BF16, kind="ExternalInput")
    wq_e = nc.dram_tensor("wqT", [D, ELOC], BF16, kind="ExternalInput")
    wk_e = nc.dram_tensor("wkT", [D, HD], BF16, kind="ExternalInput")
    wv_e = nc.dram_tensor("wvT", [D, HD], BF16, kind="ExternalInput")
    wo_e = nc.dram_tensor("woT", [D, ELOC], BF16, kind="ExternalInput")
    cos_e = nc.dram_tensor("cosT", [HD, S], BF16, kind="ExternalInput")
    sin_e = nc.dram_tensor("sinT", [HD, S], BF16, kind="ExternalInput")
    out_e = nc.dram_tensor("out", [ELOC, S], F32, kind="ExternalOutput")

    xT = xT_e.ap().rearrange("(ko p) s -> p ko s", p=128)       # [128, 32, 2048]
    wqT = wq_e.ap().rearrange("(ko p) m -> p ko m", p=128)      # [128, 32, 512]
    wkT = wk_e.ap().rearrange("(ko p) m -> p ko m", p=128)      # [128, 32, 128]
    wvT = wv_e.ap().rearrange("(ko p) m -> p ko m", p=128)
    woT = wo_e.ap().rearrange("(ko p) m -> p ko m", p=128)      # [128, 32, 512]

    rg = [list(range(NCORES))]

    with tile.TileContext(nc) as tc:
        with (
            tc.tile_pool(name="dram", bufs=1, space="DRAM") as dram_pool,
            tc.tile_pool(name="persist", bufs=1) as pp,
        ):
            # AllGather buffers: one 2-chunk AG for s-chunks 0+1 (its
            # staging completes earliest under the A/B chunk interleave),
            # then one per chunk for 2 and 3 so each triggers as soon as its
            # staging lands.
            AG_WIDTHS = [2 * CHUNK, CHUNK, CHUNK]
            ag_in = [
                dram_pool.tile([ELOC, w], BF16, name=f"ag_in{k}")
                for k, w in enumerate(AG_WIDTHS)
            ]
            ag_out = [
                dram_pool.tile(
                    [NCORES * ELOC, w], BF16, name=f"ag_out{k}",
                    addr_space="Shared",
                )
                for k, w in enumerate(AG_WIDTHS)
            ]

            def ag_slot(j):
                """(ag index, column offset) for s-chunk j."""
                return (0, j * CHUNK) if j < 2 else (j - 1, 0)

            # ---- small constants ----
            # band[p, c, u] = 1 iff u >= p + 384, duplicated at c=0,1 so a
            # head-pair exp tile can be masked with one physical (non-
            # broadcast) multiply.
            band = pp.tile([128, 2, 896], BF16)
            nc.gpsimd.memset(band[:], 1.0)
            for c in range(2):
                nc.gpsimd.affine_select(
                    out=band[:, c, :], in_=band[:, c, :],
                    compare_op=mybir.AluOpType.is_ge, fill=0.0,
                    base=-384, channel_multiplier=-1, pattern=[[1, 896]],
                )
            ones_sb = pp.tile([128, 1], BF16)
            nc.gpsimd.memset(ones_sb[:], 1.0)
            ident = pp.tile([128, 128], BF16)
            make_identity(nc, ident[:])

            cos_sb = pp.tile([128, S], BF16)
            sin_sb = pp.tile([128, S], BF16)

            # ---- persistent activations ----
            qsb = pp.tile([128, HLOC, S], BF16)     # roped qT per head
            ksb = pp.tile([128, S], BF16)           # roped kT
            vsb = pp.tile([128, NB, HD], BF16)      # v[s2-tile, :, hd]

            with (
                tc.tile_pool(name="wq", bufs=1) as wqp,
                tc.tile_pool(name="wkv", bufs=1) as wkvp,
                tc.tile_pool(name="xch", bufs=1) as xp,
                tc.tile_pool(name="rope", bufs=2) as rp,
                tc.tile_pool(name="pt", bufs=5) as ptp,
                tc.tile_pool(name="acc", bufs=2) as accp,
                tc.tile_pool(name="misc", bufs=2) as mp,
                tc.tile_pool(name="stage", bufs=3) as stp,
            ):
                def load_xchunk(j):
                    js = slice(j * CHUNK, (j + 1) * CHUNK)
                    xc_g = []
                    for g in range(4):
                        t = xp.tile(
                            [128, 8, CHUNK], BF16, tag=f"xc{g}", bufs=2,
                            name=f"xc{j}_{g}",
                        )
                        eng = nc.gpsimd if g % 2 else nc.sync
                        for s0 in range(0, 8, 4):
                            eng.dma_start(
                                t[:, s0:s0 + 4, :],
                                xT[:, 8 * g + s0:8 * g + s0 + 4, js],
                            )
                        xc_g.append(t)
                    return xc_g

                def grp_load(pool, dram_t, m, name, eng, nsplit=2):
                    tiles = []
                    for g in range(4):
                        t = pool.tile([128, 8, m], BF16, name=f"{name}{g}")
                        step = 8 // nsplit
                        for s0 in range(0, 8, step):
                            eng.dma_start(
                                t[:, s0:s0 + step, :],
                                dram_t[:, 8 * g + s0:8 * g + s0 + step, :],
                            )
                        tiles.append(t)
                    return tiles

                # DMA queue plan at startup: wk FIRST on gpsimd (the first
                # matmuls need it), x split sync/gpsimd, the 4MB wq stream on
                # the otherwise-idle ACT queue so neither x nor wk sits
                # behind it, cos/sin on sync right after x chunk 0.
                wk_g = grp_load(wkvp, wkT, HD, "wk", nc.gpsimd, nsplit=2)
                xc0_g = load_xchunk(0)
                wq_g = grp_load(wqp, wqT, ELOC, "wq", nc.scalar, nsplit=2)
                for g in range(4):
                    sl = slice(g * 512, (g + 1) * 512)
                    nc.sync.dma_start(cos_sb[:, sl], cos_e.ap()[:, sl])
                    nc.sync.dma_start(sin_sb[:, sl], sin_e.ap()[:, sl])
                wv_g = grp_load(wkvp, wvT, HD, "wv", nc.gpsimd, nsplit=2)

                def rope(dst01, src_ps, js):
                    """dst01: (ap_lo, ap_hi) bf16 targets [64, 512] each.
                    src_ps: [128, 512] psum holding permuted projection."""
                    tc_t = rp.tile([128, CHUNK], F32, tag="ropec")
                    ts_t = rp.tile([128, CHUNK], F32, tag="ropes")
                    sw_t = rp.tile([128, CHUNK], F32, tag="ropew")
                    nc.vector.tensor_mul(tc_t[:], src_ps[:], cos_sb[:, js])
                    nc.vector.tensor_mul(ts_t[:], src_ps[:], sin_sb[:, js])
                    nc.sync.dma_start(sw_t[0:64, :], ts_t[64:128, :])
                    nc.sync.dma_start(sw_t[64:128, :], ts_t[0:64, :])
                    nc.vector.tensor_sub(dst01[0], tc_t[0:64, :], sw_t[0:64, :])
                    nc.vector.tensor_add(dst01[1], tc_t[64:128, :], sw_t[64:128, :])

                # ---- phases A+B interleaved per s-chunk: projections for
                # chunk j, then attention for chunk j (which only needs
                # k/v chunks <= j). Attention staging therefore completes
                # ~150us earlier, hiding the ~55-65us AllGather latencies
                # entirely behind remaining projection/attention work.
                # One shared 8-bank PSUM pool: kq(2) + sc(2x2) + o(2); the
                # denominator tiles borrow the kq slots (A and B never use
                # them at the same time).
                SKEW = 2
                staging_last = {}
                xcs = {}
                with tc.tile_pool(name="psAB", bufs=1, space="PSUM") as ps:
                    def emit_A(j):
                        js = slice(j * CHUNK, (j + 1) * CHUNK)
                        xc_g = xcs[j] = xc0_g if j == 0 else load_xchunk(j)

                        # k, v, then q projections (v first so its DMA
                        # transposes land well before B(j)'s first PV)
                        k_ps = ps.tile([128, CHUNK], F32, tag="kq", bufs=2,
                                       name=f"kps_{j}")
                        for ko in range(NKO):
                            nc.tensor.matmul(
                                k_ps[:],
                                wk_g[ko // 8][:, ko % 8, :],
                                xc_g[ko // 8][:, ko % 8, :],
                                start=(ko == 0), stop=(ko == NKO - 1),
                            )
                        rope((ksb[0:64, js], ksb[64:128, js]), k_ps, js)

                        v_ps = ps.tile([128, CHUNK], F32, tag="kq", bufs=2,
                                       name=f"vps_{j}")
                        for ko in range(NKO):
                            nc.tensor.matmul(
                                v_ps[:],
                                wv_g[ko // 8][:, ko % 8, :],
                                xc_g[ko // 8][:, ko % 8, :],
                                start=(ko == 0), stop=(ko == NKO - 1),
                            )
                        vT_sb = mp.tile([128, CHUNK], BF16, tag="vtsb",
                                        name=f"vt_{j}")
                        nc.scalar.activation(
                            vT_sb[:], v_ps[:], mybir.ActivationFunctionType.Copy
                        )
                        for t in range(4):
                            nc.sync.dma_start_transpose(
                                vsb[:, 4 * j + t, :],
                                vT_sb[:, t * 128:(t + 1) * 128],
                            )

                        for h in range(HLOC):
                            q_ps = ps.tile([128, CHUNK], F32, tag="kq", bufs=2,
                                           name=f"qps_{j}_{h}")
                            for ko in range(NKO):
                                nc.tensor.matmul(
                                    q_ps[:],
                                    wq_g[ko // 8][:, ko % 8, h * 128:(h + 1) * 128],
                                    xc_g[ko // 8][:, ko % 8, :],
                                    start=(ko == 0), stop=(ko == NKO - 1),
                                )
                            rope((qsb[0:64, h, js], qsb[64:128, h, js]), q_ps, js)

                    def emit_B(j):
                        nblk = 4 * (j + 1)
                        for hp in range(2):
                            ngrp = j + 1     # groups of 4 s2-blocks
                            o = {}
                            dn = {}
                            for hh in range(2):
                                o[hh] = ps.tile(
                                    [128, CHUNK], F32, tag="o", bufs=2,
                                    name=f"o_{j}_{hp}_{hh}",
                                )
                                dn[hh] = ps.tile(
                                    [128, CHUNK], F32, tag="kq", bufs=2,
                                    name=f"dn_{j}_{hp}_{hh}",
                                )
                            pts = {}
                            # per-4-block bf16 exp sums (short independent
                            # DVE chains; the denominator then needs only
                            # ngrp short accumulating matmuls per head)
                            grp = {}

                            def issue_sc(i, j=j, hp=hp, pts=pts, grp=grp):
                                # Diagonal blocks at offset t>=1 have columns
                                # < 128*t fully masked: trim the matmul/exp
                                # free dim to the valid range.
                                t = i - 4 * j
                                lo = 128 * t if t >= 1 else 0
                                sc = ps.tile(
                                    [128, 2, CHUNK], F32, tag="sc", bufs=2,
                                    name=f"sc_{j}_{hp}_{i}",
                                )
                                for hh in range(2):
                                    nc.tensor.matmul(
                                        sc[:, hh, lo:CHUNK],
                                        ksb[:, i * 128:(i + 1) * 128],
                                        qsb[:, 2 * hp + hh,
                                            j * CHUNK + lo:(j + 1) * CHUNK],
                                        start=True, stop=True,
                                    )
                                pt = ptp.tile(
                                    [128, 2, CHUNK], BF16, tag="pt",
                                    name=f"pt_{j}_{hp}_{i}",
                                )
                                nc.scalar.activation(
                                    pt[:, :, lo:CHUNK], sc[:, :, lo:CHUNK],
                                    mybir.ActivationFunctionType.Exp,
                                    scale=SCALE,
                                )
                                if t >= 0:  # diagonal block: zero s1 < s2
                                    nc.vector.tensor_mul(
                                        pt[:, :, lo:CHUNK], pt[:, :, lo:CHUNK],
                                        band[:, :, 384:896 - lo],
                                    )
                                g = i // 4
                                if i % 4 == 0:
                                    grp[g] = (pt, lo, None)
                                elif i % 4 == 1:
                                    # first add CREATES the group sum in a
                                    # fresh tile (pt tiles stay pristine for
                                    # their PV matmuls)
                                    pt0, lo0, _ = grp[g]
                                    acc = ptp.tile(
                                        [128, 2, CHUNK], BF16, tag="gacc",
                                        bufs=2, name=f"gacc_{j}_{hp}_{g}",
                                    )
                                    nc.vector.tensor_add(
                                        acc[:, :, lo:CHUNK],
                                        pt0[:, :, lo:CHUNK],
                                        pt[:, :, lo:CHUNK],
                                    )
                                    if lo > lo0:
                                        nc.vector.tensor_copy(
                                            acc[:, :, lo0:lo],
                                            pt0[:, :, lo0:lo],
                                        )
                                    grp[g] = (pt0, lo0, acc)
                                else:
                                    _, _, acc = grp[g]
                                    nc.vector.tensor_add(
                                        acc[:, :, lo:CHUNK],
                                        acc[:, :, lo:CHUNK],
                                        pt[:, :, lo:CHUNK],
                                    )
                                pts[i] = (pt, lo)

                            for i in range(min(SKEW, nblk)):
                                issue_sc(i)
                            for i in range(nblk):
                                if i + SKEW < nblk:
                                    issue_sc(i + SKEW)
                                pt, lo = pts.pop(i)
                                # both heads' PV share lhsT=vsb[:,i,:]: back
                                # to back with one weight tile
                                for hh in range(2):
                                    nc.tensor.matmul(
                                        o[hh][:, lo:CHUNK], vsb[:, i, :],
                                        pt[:, hh, lo:CHUNK],
                                        start=(i == 0), stop=(i == nblk - 1),
                                    )
                                if i % 4 == 3:
                                    g = i // 4
                                    _, _, acc = grp.pop(g)
                                    for hh in range(2):
                                        nc.tensor.matmul(
                                            dn[hh][0:1, :], ones_sb[:],
                                            acc[:, hh, :],
                                            start=(g == 0),
                                            stop=(g == ngrp - 1),
                                        )
                            # evacuate psum fast, then normalize + stage
                            # (staging on the gpsimd DMA queue: the sync
                            # queue carries x/agsb/wo bulk loads)
                            for hh in range(2):
                                h = 2 * hp + hh
                                oun = mp.tile([128, CHUNK], F32, tag="oun",
                                              name=f"oun_{j}_{h}")
                                nc.vector.tensor_copy(oun[:], o[hh][:])
                                recip = mp.tile([1, CHUNK], F32, tag="recip",
                                                name=f"rc_{j}_{h}")
                                nc.vector.reciprocal_approx_fast(
                                    recip[:], dn[hh][0:1, :]
                                )
                                rb = mp.tile([128, CHUNK], F32, tag="rb",
                                             name=f"rb_{j}_{h}")
                                nc.gpsimd.partition_broadcast(rb[:], recip[:])
                                att = stp.tile([128, CHUNK], BF16, tag="att",
                                               name=f"att_{j}_{h}")
                                nc.vector.tensor_mul(att[:], oun[:], rb[:])
                                k, co = ag_slot(j)
                                last = nc.gpsimd.dma_start(
                                    ag_in[k][h * 128:(h + 1) * 128,
                                             co:co + CHUNK],
                                    att[:],
                                )
                                staging_last[j] = last
                        if j >= 1:
                            k = ag_slot(j)[0]
                            nc.gpsimd.collective_compute(
                                "AllGather",
                                mybir.AluOpType.bypass,
                                replica_groups=rg,
                                ins=[ag_in[k][:].opt()],
                                outs=[ag_out[k][:].opt()],
                            )

                    # Schedule: A0 A1 B0 A2 B1 A3 B2 B3. Projections run two
                    # chunks ahead of attention so each rope evacuation
                    # enqueues on the (in-order) DVE queue BEFORE the next
                    # B-phase's exp-gated ops — otherwise the kq-psum WAR
                    # release for A(j)'s matmuls waits out an entire B phase
                    # (measured 28us PE stall + p-state downshift).
                    emit_A(0)
                    emit_A(1)
                    for j in range(NCHUNK):
                        emit_B(j)
                        if j + 2 < NCHUNK:
                            emit_A(j + 2)
                    last_attn_inst = staging_last[NCHUNK - 1]

            # ---- phase C: output projection (column-parallel) ----
            with (
                tc.tile_pool(name="wo", bufs=1) as wop,
                tc.tile_pool(name="ag", bufs=2) as agp,
                tc.tile_pool(name="ost", bufs=3) as ostp,
                tc.tile_pool(name="psC", bufs=2, space="PSUM") as psc,
            ):
                wo_sb = wop.tile([128, NKO, ELOC], BF16)
                for g in range(8):
                    ko = slice(4 * g, 4 * g + 4)
                    nc.sync.dma_start(wo_sb[:, ko, :], woT[:, ko, :])
                first_wo = True
                for j in range(NCHUNK):
                    js = slice(j * CHUNK, (j + 1) * CHUNK)
                    k, co = ag_slot(j)
                    agt = ag_out[k][:].rearrange("(ko p) s -> p ko s", p=128)
                    # split the gathered-attn load into 4 e-groups so the first
                    # wo matmuls start after ~1MB instead of 4MB of DMA
                    aggrp = []
                    for g in range(4):
                        agsb = agp.tile(
                            [128, 8, CHUNK], BF16, tag=f"agsb{g}", bufs=2
                        )
                        # These wait on their AllGather's semaphore inside
                        # the sync FIFO; nothing latency-critical sits behind
                        # them there (staging runs on gpsimd), and chunks 0/1
                        # prefetch during B(2)/B(3) once AG0 lands.
                        nc.sync.dma_start(
                            agsb[:, 0:4, :],
                            agt[:, 8 * g:8 * g + 4, co:co + CHUNK],
                        )
                        nc.sync.dma_start(
                            agsb[:, 4:8, :],
                            agt[:, 8 * g + 4:8 * g + 8, co:co + CHUNK],
                        )
                        aggrp.append(agsb)
                    for t in range(4):
                        wo_ps = psc.tile([128, CHUNK], F32, tag="wo")
                        for ko in range(NKO):
                            mm = nc.tensor.matmul(
                                wo_ps[:],
                                wo_sb[:, ko, t * 128:(t + 1) * 128],
                                aggrp[ko // 8][:, ko % 8, :],
                                start=(ko == 0), stop=(ko == NKO - 1),
                            )
                            if first_wo:
                                # keep the PE stream ordered: all attention
                                # before any wo
                                tile.add_dep_helper(
                                    mm.ins, last_attn_inst.ins, sync=False,
                                    reason="attention before wo on PE",
                                )
                                first_wo = False
                        osb = ostp.tile([128, CHUNK], F32, tag="osb")
                        if j == 3 and t == 3:
                            # last tile: evacuate on the idle DVE (skips the
                            # ACT queue) and fan the write across both DMA
                            # pools to shorten the kernel tail
                            nc.vector.tensor_copy(osb[:], wo_ps[:])
                            engs = [nc.sync, nc.gpsimd, nc.sync, nc.gpsimd]
                            for q in range(4):
                                c0 = 128 * q
                                engs[q].dma_start(
                                    out_e.ap()[t * 128:(t + 1) * 128,
                                               js.start + c0:js.start + c0 + 128],
                                    osb[:, c0:c0 + 128],
                                )
                        else:
                            nc.scalar.activation(
                                osb[:], wo_ps[:],
                                mybir.ActivationFunctionType.Copy
                            )
                            nc.gpsimd.dma_start(
                                out_e.ap()[t * 128:(t + 1) * 128,
                                           js.start:js.start + 256],
                                osb[:, 0:256],
                            )
                            nc.gpsimd.dma_start(
                                out_e.ap()[t * 128:(t + 1) * 128,
                                           js.start + 256:js.stop],
                                osb[:, 256:CHUNK],
                            )

    nc.compile()
    return nc


def _get_nc():
    global _NC_CACHE
    if _NC_CACHE is None:
        _NC_CACHE = _build()
    return _NC_CACHE


_PERM = np.concatenate([np.arange(0, HD, 2), np.arange(1, HD, 2)])


def _prep_inputs(x, freqs_cos, freqs_sin, wq, wk, wv, wo):
    xT = np.ascontiguousarray(x.reshape(S, D).T.astype(bf16))
    cosT = np.ascontiguousarray(
        np.concatenate([freqs_cos.T, freqs_cos.T], axis=0).astype(bf16)
    )
    sinT = np.ascontiguousarray(
        np.concatenate([freqs_sin.T, freqs_sin.T], axis=0).astype(bf16)
    )
    in_maps = []
    for c in range(NCORES):
        heads = range(HLOC * c, HLOC * (c + 1))
        wq_c = np.concatenate(
            [wq[h * HD:(h + 1) * HD][_PERM] for h in heads], axis=0
        )  # [512, D] permuted
        wqT_c = np.ascontiguousarray(wq_c.T.astype(bf16))
        wk_c = wk[c * HD:(c + 1) * HD][_PERM]
        wkT_c = np.ascontiguousarray(wk_c.T.astype(bf16))
        wv_c = wv[c * HD:(c + 1) * HD]
        wvT_c = np.ascontiguousarray(wv_c.T.astype(bf16))
        woT_c = np.ascontiguousarray(wo[c * ELOC:(c + 1) * ELOC, :].T.astype(bf16))
        in_maps.append(
            {
                "xT": xT,
                "wqT": wqT_c,
                "wkT": wkT_c,
                "wvT": wvT_c,
                "woT": woT_c,
                "cosT": cosT,
                "sinT": sinT,
            }
        )
    return in_maps


def _run(in_maps, trace=False, trace_cores=None):
    nc = _get_nc()
    return run_bass_kernel_spmd(
        nc,
        in_maps,
        list(range(NCORES)),
        trace=trace,
        trace_cores=trace_cores,
    )


def kernel(x, freqs_cos, freqs_sin, wq, wk, wv, wo):
    x = np.asarray(x, dtype=np.float32)
    in_maps = _prep_inputs(
        x,
        np.asarray(freqs_cos, np.float32),
        np.asarray(freqs_sin, np.float32),
        np.asarray(wq, np.float32),
        np.asarray(wk, np.float32),
        np.asarray(wv, np.float32),
        np.asarray(wo, np.float32),
    )
    res = _run(in_maps)
    out = np.empty((S, D), dtype=np.float32)
    for c in range(NCORES):
        out[:, c * ELOC:(c + 1) * ELOC] = np.asarray(
            res.results[c]["out"], dtype=np.float32
        ).T
    return out.reshape(B, S, D)
